# revision 3
# baseline (speedup 1.0000x reference)
"""Bass/Trainium2 kernel for a 2-block single-head causal transformer.

Strategy (8 NeuronCores): data-parallel over batch (B=4 -> 4 core pairs),
sequence-parallel within each pair. Each core owns the interleaved global
query tiles {2j + t} (t = core parity), so the instruction stream is
identical on every core; all per-core variation (tokens, positional rows,
causal edge masks, vocab slice) is input data.

Per block, each core computes K^T / V for its own rows only; the halves are
exchanged with a pair-wise AllGather (rank order == parity order, so the
"stored" key order [even tiles | odd tiles] is core-invariant). Attention,
Wo, and the FFN then run on the core's own rows without further
communication. The final-token logits are computed with the vocab sharded
8 ways after a tiny 8-core AllGather of the last-row activations.

Everything is bf16 into the PE array with fp32 PSUM accumulation; softmax
runs in fp32 on PSUM-resident scores with deferred normalization.
"""

import sys

sys.path.insert(0, "/opt/trn_rl_repo")

import numpy as np
import ml_dtypes

import concourse.bass as bass
import concourse.mybir as mybir
import concourse.tile as tile
from concourse import bacc
from concourse.bass import IndirectOffsetOnAxis
from concourse.bass_utils import run_bass_kernel_spmd
from concourse.masks import make_identity

BF16 = mybir.dt.bfloat16
F32 = mybir.dt.float32
I32 = mybir.dt.int32
P = 128
NEG = -1.0e9


def _chunks(total, step):
    out = []
    off = 0
    while off < total:
        out.append((off, min(step, total - off)))
        off += step
    return out


def build_nc(S=2048, D=1024, H=4096, V=32000, n_cores=8, stage="full"):
    """Build the SPMD Bass program (identical on all cores).

    stage: "h0" | "kv" | "attn" | "block1" | "blocks" | "full" — truncate the
    program after the named phase and dump an intermediate to `dbg` (debug).
    """
    NJ = (S // P) // 2          # own q-tiles (slots) per core
    ND = D // P                 # d blocks
    NH = H // P                 # h blocks
    SO = S // 2                 # own rows per core
    VS = V // n_cores           # vocab slice per core
    W1CH = min(8, NH)           # h-blocks per streamed w1 chunk
    QH = min(512, SO)           # q-half size for the FFN
    VC = 500 if VS % 500 == 0 else VS  # logits n-chunk
    pair_groups = [[2 * i, 2 * i + 1] for i in range(n_cores // 2)]
    all_group = [list(range(n_cores))]

    nc = bacc.Bacc("TRN2", target_bir_lowering=False, debug=False,
                   num_devices=n_cores)

    # ---- external inputs ----
    # h0_own = emb[tokens[own rows]] + pe[own rows], staged on the host as
    # part of sharding (pure row-gather data movement, no compute)
    h0_own = nc.dram_tensor("h0_own", [NJ, P, D], BF16, kind="ExternalInput")
    mask = nc.dram_tensor("mask", [NJ, P, 2 * P], BF16, kind="ExternalInput")
    wts = {}
    for l in (1, 2):
        wts[l, "wk"] = nc.dram_tensor(f"l{l}_wk", [D, D], BF16, kind="ExternalInput")
        wts[l, "wv"] = nc.dram_tensor(f"l{l}_wv", [D, D], BF16, kind="ExternalInput")
        wts[l, "wo"] = nc.dram_tensor(f"l{l}_wo", [D, D], BF16, kind="ExternalInput")
        wts[l, "w1"] = nc.dram_tensor(f"l{l}_w1", [D, H], BF16, kind="ExternalInput")
        wts[l, "w2"] = nc.dram_tensor(f"l{l}_w2", [H, D], BF16, kind="ExternalInput")
    w_out = nc.dram_tensor("w_out", [D, VS], BF16, kind="ExternalInput")
    logits = nc.dram_tensor("logits", [4, VS], F32, kind="ExternalOutput")
    dbg = None
    if stage != "full":
        dbg = nc.dram_tensor("dbg", [P, ND, S], BF16, kind="ExternalOutput")

    with tile.TileContext(nc) as tc:
        with (
            tc.tile_pool(name="big", bufs=2) as big,          # kT / v / midT
            tc.tile_pool(name="own", bufs=1) as own_p,        # own hT
            tc.tile_pool(name="hat", bufs=1) as hat_p,        # h_attnT
            tc.tile_pool(name="res", bufs=1) as res_p,        # h_resT
            tc.tile_pool(name="w", bufs=2) as w_p,            # streamed weights
            tc.tile_pool(name="attn", bufs=2) as attn_p,
            tc.tile_pool(name="attnT", bufs=2) as attnT_p,
            tc.tile_pool(name="small", bufs=4) as small_p,    # evict staging
            tc.tile_pool(name="misc", bufs=2) as misc_p,
            tc.tile_pool(name="const", bufs=1) as const_p,
            tc.tile_pool(name="ps_mm", bufs=2, space="PSUM") as ps_mm,
            tc.tile_pool(name="ps_sc", bufs=1, space="PSUM") as ps_sc,
            tc.tile_pool(name="ps_av", bufs=1, space="PSUM") as ps_av_p,
            tc.tile_pool(name="dram", bufs=2, space="DRAM") as dram_p,
        ):
            mask_sb = const_p.tile([P, NJ, 2 * P], BF16, tag="mask")
            nc.sync.dma_start(mask_sb[:], mask[:].rearrange("j p c -> p j c"))

            ident = const_p.tile([P, P], BF16, tag="ident")
            make_identity(nc, ident[:])

            def pe_transpose(dst_ap, src_ap):
                # PE transpose (128x128 bf16) + DVE copy back to SBUF
                pst = ps_mm.tile([P, P], BF16, tag="mm")
                nc.tensor.transpose(pst[:], src_ap, ident[:])
                nc.vector.tensor_copy(dst_ap, pst[:])

            def logits_prologue():
                # 8-core AllGather of the last token's activations; transpose
                # [128p, ND] -> [ND, 128] first so the DRAM write (and the
                # gathered reads) are contiguous in d-order. Only the AllGather
                # is issued here; the consumers are emitted after the FFN so
                # the collective latency hides under the remaining FFN work.
                lc_t = misc_p.tile([ND, P], BF16, tag="lct")
                ps_lc = ps_mm.tile([P, 512], BF16, tag="mm")
                nc.tensor.transpose(ps_lc[:ND, :P], last_col[:], ident[:])
                nc.vector.tensor_copy(lc_t[:], ps_lc[:ND, :P])
                cc_l_in = dram_p.tile([D], BF16, tag="ccl")
                cc_l_out = dram_p.tile([n_cores, D], BF16, tag="cclo")
                nc.sync.dma_start(cc_l_in[:].rearrange("(i p) -> i p", p=P), lc_t[:])
                nc.gpsimd.collective_compute(
                    "AllGather", mybir.AluOpType.bypass,
                    replica_groups=all_group,
                    ins=[cc_l_in[:].opt()], outs=[cc_l_out[:].opt()],
                )
                return cc_l_out

            def logits_lhsT(cc_l_out):
                # rows 1,3,5,7 hold batches 0..3 (odd cores own the last row)
                h_last = misc_p.tile([4, ND, P], BF16, tag="hlast")
                nc.sync.dma_start(
                    h_last[:],
                    cc_l_out[:].rearrange("r (i p) -> r i p", p=P)[1::2],
                )
                lhsT = const_p.tile([P, ND, 4], BF16, tag="lhsT")
                for i in range(ND):
                    ps_t = ps_mm.tile([P, 512], BF16, tag="mm")
                    nc.tensor.transpose(ps_t[:, :4], h_last[:, i, :], ident[:4, :4])
                    nc.vector.tensor_copy(lhsT[:, i, :], ps_t[:, :4])
                return lhsT

            # ---------------- h0 load + transpose ----------------
            own_hT = own_p.tile([P, ND, SO], BF16, tag="own")
            for j in range(NJ):
                h0 = misc_p.tile([P, D], BF16, tag="h0", bufs=4)
                eng = nc.sync if j % 2 == 0 else nc.scalar
                eng.dma_start(h0[:], h0_own[j])
                for i in range(ND):
                    pe_transpose(
                        own_hT[:, i, j * P : (j + 1) * P],
                        h0[:, i * P : (i + 1) * P],
                    )

            if stage == "h0":
                nc.sync.dma_start(dbg[:, :, :SO], own_hT[:])

            # ---------------- transformer blocks ----------------
            if stage == "h0":
                blocks = ()
            elif stage in ("kv", "attn", "block1"):
                blocks = (1,)
            else:
                blocks = (1, 2)
            for l in blocks:
                wk_sb = w_p.tile([P, ND, D], BF16, tag="w")
                nc.sync.dma_start(wk_sb[:], wts[l, "wk"][:].rearrange("(k p) n -> p k n", p=P))
                wv_sb = w_p.tile([P, ND, D], BF16, tag="w")
                nc.sync.dma_start(wv_sb[:], wts[l, "wv"][:].rearrange("(k p) n -> p k n", p=P))

                # K^T / V for own rows, AllGathered in column/row halves so
                # attention can start after the first half lands.
                KH = SO // 2            # own-column half for the k AllGather
                MH = NJ // 2            # own s-tiles per v AllGather half
                cc_in_k = [dram_p.tile([D, KH], BF16, tag=f"cck{h}", name=f"cck{h}") for h in range(2)]
                cc_out_k = [dram_p.tile([2, D, KH], BF16, tag=f"ccko{h}", name=f"ccko{h}") for h in range(2)]
                cc_in_v = [dram_p.tile([MH * P, D], BF16, tag=f"ccv{h}", name=f"ccv{h}") for h in range(2)]
                cc_out_v = [dram_p.tile([2, MH * P, D], BF16, tag=f"ccvo{h}", name=f"ccvo{h}") for h in range(2)]

                for hh in range(2):
                    # kT_own columns [hh*KH : (hh+1)*KH]
                    for off0, n in _chunks(KH, 512):
                        off = hh * KH + off0
                        for i in range(ND):
                            ps = ps_mm.tile([P, 512], F32, tag="mm")
                            for k in range(ND):
                                nc.tensor.matmul(
                                    ps[:, :n],
                                    wk_sb[:, k, i * P : (i + 1) * P],
                                    own_hT[:, k, off : off + n],
                                    start=(k == 0),
                                    stop=(k == ND - 1),
                                )
                            st = small_p.tile([P, 512], BF16, tag="st")
                            nc.vector.tensor_copy(st[:, :n], ps[:, :n])
                            eng = nc.sync if i % 2 == 0 else nc.scalar
                            eng.dma_start(
                                cc_in_k[hh][i * P : (i + 1) * P, off0 : off0 + n],
                                st[:, :n],
                            )
                    nc.gpsimd.collective_compute(
                        "AllGather", mybir.AluOpType.bypass,
                        replica_groups=pair_groups,
                        ins=[cc_in_k[hh][:].opt()], outs=[cc_out_k[hh][:].opt()],
                    )

                for hh in range(2):
                    # v_own s-tiles [hh*MH : (hh+1)*MH]
                    for m0 in range(MH):
                        m = hh * MH + m0
                        for off, n in _chunks(D, 512):
                            ps = ps_mm.tile([P, 512], F32, tag="mm")
                            for k in range(ND):
                                nc.tensor.matmul(
                                    ps[:, :n],
                                    own_hT[:, k, m * P : (m + 1) * P],
                                    wv_sb[:, k, off : off + n],
                                    start=(k == 0),
                                    stop=(k == ND - 1),
                                )
                            st = small_p.tile([P, 512], BF16, tag="st")
                            nc.vector.tensor_copy(st[:, :n], ps[:, :n])
                            eng = nc.sync if m % 2 == 0 else nc.scalar
                            eng.dma_start(
                                cc_in_v[hh][m0 * P : (m0 + 1) * P, off : off + n],
                                st[:, :n],
                            )
                    nc.gpsimd.collective_compute(
                        "AllGather", mybir.AluOpType.bypass,
                        replica_groups=pair_groups,
                        ins=[cc_in_v[hh][:].opt()], outs=[cc_out_v[hh][:].opt()],
                    )

                # stored-key quarters: q = 2*rank + half, columns [P, ND, KH]
                # load order (0,2,1,3): the quarters fed by the first
                # AllGather go into the FIFO DMA queues first, so the second
                # AllGather's waits don't block them
                kT_q = [None] * 4
                for q in (0, 2, 1, 3):
                    r, hh = q // 2, q % 2
                    t = big.tile([P, ND, KH], BF16, tag=f"kv{q}", name=f"kTq{q}")
                    for i in range(ND):
                        eng = nc.sync if i % 2 == 0 else nc.scalar
                        eng.dma_start(
                            t[:, i, :], cc_out_k[hh][r, i * P : (i + 1) * P, :],
                        )
                    kT_q[q] = t

                # stored-v quarters: q covers stored s-tiles [q*MH:(q+1)*MH]
                v_q = [None] * 4
                for q in (0, 2, 1, 3):
                    r, hh = q // 2, q % 2
                    t = big.tile([P, MH, D], BF16, tag=f"kv{q}", name=f"vq{q}")
                    for m0 in range(MH):
                        eng = nc.sync if m0 % 2 == 0 else nc.scalar
                        eng.dma_start(
                            t[:, m0, :], cc_out_v[hh][r, m0 * P : (m0 + 1) * P, :],
                        )
                    v_q[q] = t

                if stage == "kv":
                    for q in range(4):
                        nc.sync.dma_start(
                            dbg[:, :, q * KH : (q + 1) * KH], kT_q[q][:],
                        )
                    break

                # prefetch wo while attention runs
                wo_sb = w_p.tile([P, ND, D], BF16, tag="w")
                nc.sync.dma_start(wo_sb[:], wts[l, "wo"][:].rearrange("(k p) n -> p k n", p=P))

                h_attnT = hat_p.tile([P, ND, SO], BF16, tag="hat")

                # ---------------- attention, software-pipelined over slots:
                # slot j's scores/softmax overlap slot j-1's attn@v on PE
                pend = {}

                def attn_head(j):
                    W1 = P * (j + 1)
                    ps_s = ps_sc.tile([P, S], F32, tag="sc")
                    # scores: two ranges (rank0 keys at [0:W1], rank1 at [SO:])
                    for base in (0, SO):
                        for off, n in _chunks(W1, min(512, KH)):
                            q = 2 * (base // SO) + (off // KH)
                            lo = off % KH
                            for k in range(ND):
                                nc.tensor.matmul(
                                    ps_s[:, base + off : base + off + n],
                                    own_hT[:, k, j * P : (j + 1) * P],
                                    kT_q[q][:, k, lo : lo + n],
                                    start=(k == 0),
                                    stop=(k == ND - 1),
                                )
                    # causal edge masks (one edge tile per range)
                    nc.vector.tensor_add(
                        ps_s[:, W1 - P : W1], ps_s[:, W1 - P : W1],
                        mask_sb[:, j, 0:P],
                    )
                    nc.vector.tensor_add(
                        ps_s[:, SO + W1 - P : SO + W1],
                        ps_s[:, SO + W1 - P : SO + W1],
                        mask_sb[:, j, P : 2 * P],
                    )
                    # softmax over both ranges (3D AP [P, 2, W1])
                    sc2 = ps_s[:].rearrange("p (r s) -> p r s", s=SO)[:, :, :W1]
                    negmax = misc_p.tile([P, 1], F32, tag="negmax")
                    nc.vector.reduce_max(negmax[:], sc2, axis=mybir.AxisListType.XY,
                                         negate=True)
                    attn = attn_p.tile([P, S], BF16, tag="attn")
                    att2 = attn[:].rearrange("p (r s) -> p r s", s=SO)[:, :, :W1]
                    lsum = misc_p.tile([P, 1], F32, tag="lsum")
                    nc.scalar.activation(att2, sc2, mybir.ActivationFunctionType.Exp,
                                         bias=negmax[:], scale=1.0, accum_out=lsum[:])
                    inv_l = misc_p.tile([P, 1], F32, tag="invl")
                    nc.vector.reciprocal(inv_l[:], lsum[:])
                    pend[j] = (attn, inv_l)

                def attn_tail(j):
                    attn, inv_l = pend.pop(j)
                    attnT = attnT_p.tile([P, 2 * NJ, P], BF16, tag="attnT")
                    for r in range(2):
                        for kk in range(j + 1):
                            pe_transpose(
                                attnT[:, r * NJ + kk, :],
                                attn[:, r * SO + kk * P : r * SO + (kk + 1) * P],
                            )
                    # attn @ v -> h_attn [q, D] (natural), deferred 1/l scale
                    ps_av = ps_av_p.tile([P, 1024], F32, tag="av")
                    for off, n in _chunks(D, 512):
                        first = True
                        for r in range(2):
                            for kk in range(j + 1):
                                g = r * NJ + kk
                                nc.tensor.matmul(
                                    ps_av[:, off : off + n],
                                    attnT[:, g, :],
                                    v_q[g // MH][:, g % MH, off : off + n],
                                    start=first,
                                    stop=(r == 1 and kk == j),
                                )
                                first = False
                    h_attn = misc_p.tile([P, D], BF16, tag="hattn")
                    nc.vector.tensor_scalar_mul(h_attn[:], ps_av[:, :D], inv_l[:])
                    # transpose into h_attnT columns for this slot
                    for i in range(ND):
                        pe_transpose(
                            h_attnT[:, i, j * P : (j + 1) * P],
                            h_attn[:, i * P : (i + 1) * P],
                        )

                for j in range(NJ):
                    attn_head(j)
                    if j > 0:
                        attn_tail(j - 1)
                attn_tail(NJ - 1)

                if stage == "attn":
                    nc.sync.dma_start(dbg[:, :, :SO], h_attnT[:])
                    break

                # ---------------- Wo + residual -> h_resT
                h_resT = res_p.tile([P, ND, SO], BF16, tag="res")
                for i in range(ND):
                    for off, n in _chunks(SO, 512):
                        ps = ps_mm.tile([P, 512], F32, tag="mm")
                        for k in range(ND):
                            nc.tensor.matmul(
                                ps[:, :n],
                                wo_sb[:, k, i * P : (i + 1) * P],
                                h_attnT[:, k, off : off + n],
                                start=(k == 0),
                                stop=(k == ND - 1),
                            )
                        nc.vector.tensor_add(
                            h_resT[:, i, off : off + n], ps[:, :n],
                            own_hT[:, i, off : off + n],
                        )

                # ---------------- FFN (per q-half; streamed w1/w2)
                # block 2 runs halves in reverse so the half holding the
                # final token finishes first -> the logits AllGather and
                # w_out streaming overlap the remaining FFN work
                own_hT_next = own_p.tile([P, ND, SO], BF16, tag="own")
                if l == 2 and stage == "full":
                    last_col = misc_p.tile([P, ND], BF16, tag="lastcol")
                n_w1ch = (NH + W1CH - 1) // W1CH
                qchunks = _chunks(SO, QH)
                if l == 2:
                    qchunks = qchunks[::-1]
                NHQ = NH // 4           # h-blocks per midT quarter
                for qoff, qn in qchunks:
                    midT = [big.tile([P, NHQ, QH], BF16, tag=f"kv{q}", name=f"midT{q}")
                            for q in range(4)]
                    for ch in range(n_w1ch):
                        hb0 = ch * W1CH
                        nhb = min(W1CH, NH - hb0)
                        w1_sb = w_p.tile([P, ND, W1CH * P], BF16, tag="w")
                        nc.sync.dma_start(
                            w1_sb[:, :, : nhb * P],
                            wts[l, "w1"][:, hb0 * P : (hb0 + nhb) * P]
                            .rearrange("(k p) n -> p k n", p=P),
                        )
                        for hb in range(nhb):
                            g = hb0 + hb
                            ps = ps_mm.tile([P, 512], F32, tag="mm")
                            for k in range(ND):
                                nc.tensor.matmul(
                                    ps[:, :qn],
                                    w1_sb[:, k, hb * P : (hb + 1) * P],
                                    h_resT[:, k, qoff : qoff + qn],
                                    start=(k == 0),
                                    stop=(k == ND - 1),
                                )
                            nc.vector.tensor_scalar_max(
                                midT[g // NHQ][:, g % NHQ, :qn], ps[:, :qn], 0.0,
                            )
                    for i in range(ND):
                        w2_sb = w_p.tile([P, NH, P], BF16, tag="w")
                        nc.scalar.dma_start(
                            w2_sb[:],
                            wts[l, "w2"][:, i * P : (i + 1) * P]
                            .rearrange("(k p) n -> p k n", p=P),
                        )
                        ps = ps_mm.tile([P, 512], F32, tag="mm")
                        for hb in range(NH):
                            nc.tensor.matmul(
                                ps[:, :qn],
                                w2_sb[:, hb, :],
                                midT[hb // NHQ][:, hb % NHQ, :qn],
                                start=(hb == 0),
                                stop=(hb == NH - 1),
                            )
                        nc.vector.tensor_add(
                            own_hT_next[:, i, qoff : qoff + qn], ps[:, :qn],
                            h_resT[:, i, qoff : qoff + qn],
                        )
                        if l == 2 and stage == "full" and qoff + qn == SO:
                            # last token's activations, kept in a tiny tile so
                            # the logits path doesn't wait on the whole FFN
                            nc.vector.tensor_add(
                                last_col[:, i : i + 1], ps[:, qn - 1 : qn],
                                h_resT[:, i, SO - 1 : SO],
                            )
                    if l == 2 and stage == "full" and qoff + qn == SO:
                        # emit the logits AllGather prologue here so the PE
                        # transposes interleave into the remaining FFN work
                        # and the 8-core collective overlaps it
                        lgp = logits_prologue()
                own_hT = own_hT_next
                if stage == "block1":
                    nc.sync.dma_start(dbg[:, :, :SO], own_hT[:])
                    break

            if stage == "blocks":
                nc.sync.dma_start(dbg[:, :, :SO], own_hT[:])

            if stage == "full":
                lhsT = logits_lhsT(lgp)
                # w_out streamed in VC-wide chunks through the kv pool slots;
                # the first chunks' slots free mid-way through block-2 FFN so
                # most of the stream overlaps compute
                wo_ts = []
                for ci, (off, n) in enumerate(_chunks(VS, VC)):
                    wo_t = big.tile([P, ND, VC], BF16, tag=f"kv{ci % 4}", name=f"wot{ci}")
                    eng = nc.sync if ci % 2 == 0 else nc.scalar
                    eng.dma_start(
                        wo_t[:, :, :n],
                        w_out[:, off : off + n].rearrange("(k p) n -> p k n", p=P),
                    )
                    wo_ts.append(wo_t)
                for ci, (off, n) in enumerate(_chunks(VS, VC)):
                    ps = ps_mm.tile([P, 512], F32, tag="mm")
                    for k in range(ND):
                        nc.tensor.matmul(
                            ps[:4, :n], lhsT[:, k, :],
                            wo_ts[ci][:, k, :n],
                            start=(k == 0), stop=(k == ND - 1),
                        )
                    lg = misc_p.tile([4, VC], F32, tag="lg")
                    nc.vector.tensor_copy(lg[:, :n], ps[:4, :n])
                    nc.sync.dma_start(logits[:, off : off + n], lg[:, :n])

    nc.compile()
    return nc


# ----------------------------------------------------------------------------
# host side
# ----------------------------------------------------------------------------

def make_in_maps(tokens, emb, pe, weights, S=2048, D=1024, H=4096, V=32000,
                 n_cores=8):
    """weights: dict with l{1,2}_{wk,wv,wo,w1,w2} and w_out (fp32 numpy)."""
    bf = ml_dtypes.bfloat16
    NJ = (S // P) // 2
    VS = V // n_cores
    emb_f = np.ascontiguousarray(emb, dtype=np.float32)
    pe_f = np.asarray(pe, dtype=np.float32)
    scale = 1.0 / np.sqrt(float(D))
    w_bf = {}
    for l in (1, 2):
        w_bf[f"l{l}_wk"] = (np.asarray(weights[f"l{l}_wk"], np.float32) * scale).astype(bf)
        for nm in ("wv", "wo", "w1", "w2"):
            w_bf[f"l{l}_{nm}"] = np.asarray(weights[f"l{l}_{nm}"], np.float32).astype(bf)
    w_out_bf = np.asarray(weights["w_out"], np.float32).astype(bf)

    tokens = np.asarray(tokens)
    B = tokens.shape[0]
    in_maps = []
    tri = np.triu(np.full((P, P), NEG, np.float32), k=1)  # [q, k] mask
    for c in range(n_cores):
        b, t = c // 2, c % 2
        own_rows = np.concatenate(
            [np.arange((2 * j + t) * P, (2 * j + t + 1) * P) for j in range(NJ)]
        )
        tok_own = tokens[b, own_rows].astype(np.int64)
        h0_own = (emb_f[tok_own] + pe_f[own_rows]).astype(bf).reshape(NJ, P, D)
        mask = np.zeros((NJ, P, 2 * P), np.float32)
        for j in range(NJ):
            if t == 0:
                mask[j, :, :P] = tri
                mask[j, :, P:] = NEG
            else:
                mask[j, :, P:] = tri
        in_map = {
            "h0_own": h0_own,
            "mask": mask.astype(bf),
            "w_out": np.ascontiguousarray(w_out_bf[:, c * VS : (c + 1) * VS]),
        }
        in_map.update(w_bf)
        in_maps.append(in_map)
    return in_maps


_NC_CACHE = {}


def _get_nc(key=(2048, 1024, 4096, 32000, 8)):
    if key not in _NC_CACHE:
        _NC_CACHE[key] = build_nc(*key)
    return _NC_CACHE[key]


def kernel(tokens, emb, pe, l1_wk, l1_wv, l1_wo, l1_w1, l1_w2,
           l2_wk, l2_wv, l2_wo, l2_w1, l2_w2, w_out):
    S = int(np.asarray(tokens).shape[1])
    D = int(np.asarray(emb).shape[1])
    H = int(np.asarray(l1_w1).shape[1])
    V = int(np.asarray(emb).shape[0])
    n_cores = 8
    nc = _get_nc((S, D, H, V, n_cores))
    weights = dict(
        l1_wk=l1_wk, l1_wv=l1_wv, l1_wo=l1_wo, l1_w1=l1_w1, l1_w2=l1_w2,
        l2_wk=l2_wk, l2_wv=l2_wv, l2_wo=l2_wo, l2_w1=l2_w1, l2_w2=l2_w2,
        w_out=w_out,
    )
    in_maps = make_in_maps(tokens, emb, pe, weights, S, D, H, V, n_cores)
    try:
        res = run_bass_kernel_spmd(nc, in_maps, core_ids=list(range(n_cores)))
    except Exception:
        # a previous crashed run can leave the device wedged; one retry
        # (fresh NRT session) clears it
        import os
        os.environ.setdefault("NEURON_RT_RESET_CORES", "1")
        res = run_bass_kernel_spmd(nc, in_maps, core_ids=list(range(n_cores)))
    VS = V // n_cores
    out = np.zeros((np.asarray(tokens).shape[0], V), np.float32)
    for c in range(n_cores):
        out[:, c * VS : (c + 1) * VS] = res.results[c]["logits"]
    return out



# revision 8
# speedup vs baseline: 1.0421x; 1.0421x over previous
"""Bass/Trainium2 kernel for a 2-block single-head causal transformer.

Strategy (8 NeuronCores): data-parallel over batch (B=4 -> 4 core pairs),
sequence-parallel within each pair. Each core owns the interleaved global
query tiles {2j + t} (t = core parity), so the instruction stream is
identical on every core; all per-core variation (embedded rows, causal
masks, vocab slice) is input data.

v2 design (vs the first working version):
- h0 arrives from the host already transposed -> zero PE transposes at start.
- Attention computes TRANSPOSED scores  scoresT[t, q] = k_t . q_q  so the
  post-softmax attn@v contraction (over t) needs NO transposes at all, and
  softmax runs max-free (scores are bounded ~20, exp is fp32-safe), removing
  the cross-range reduce_max. The softmax denominator l[q] is recovered with
  ones-vector matmuls on the PE and folded back in via a GPSIMD
  partition_broadcast + the (mandatory anyway) PSUM->SBUF drain multiply.
- K^T / V are exchanged pair-wise in column/row halves with AllGathers
  interleaved (k0, v0, k1, v1); attention streams over stored key tiles in
  AllGather-arrival order, so collective latency hides under score compute.
- Stored-slot indexing is rank-major (slot s = r*8 + m), which makes the
  instruction stream core-invariant; the one-block causal asymmetry between
  the two parities is absorbed by per-core mask data (head-block masks).
- The final-token logits machinery (w_out streaming, logits matmuls) is
  interleaved into the block-2 FFN so only a small drain remains at the end.

Everything is bf16 into the PE array with fp32 PSUM accumulation.
"""

import sys

sys.path.insert(0, "/opt/trn_rl_repo")

import numpy as np
import ml_dtypes

import concourse.bass as bass
import concourse.mybir as mybir
import concourse.tile as tile
from concourse import bacc
from concourse.bass_utils import run_bass_kernel_spmd
from concourse.masks import make_identity

BF16 = mybir.dt.bfloat16
F32 = mybir.dt.float32
P = 128
NEG = -30000.0


def build_nc(S=2048, D=1024, H=4096, V=32000, n_cores=8, stage="full"):
    """Build the SPMD Bass program (identical on all cores).

    stage: "h0" | "kv" | "att" | "block1" | "full" — truncate after the named
    phase and dump an intermediate to `dbg` (debug).
    """
    NJ = (S // P) // 2          # own q-tiles per core
    ND = D // P                 # d blocks
    NH = H // P                 # h blocks
    SO = S // 2                 # own rows per core
    NS = 2 * NJ                 # stored key tiles (both ranks), s = r*NJ + m
    VS = V // n_cores           # vocab slice per core
    KH = SO // 2                # kT columns per AllGather half
    MH = NJ // 2                # v row-tiles per AllGather half
    W1CH = 8                    # h-blocks per streamed w1 chunk
    VC = 500                    # logits n-chunk
    NVC = VS // VC              # logits chunks (8)
    pair_groups = [[2 * i, 2 * i + 1] for i in range(n_cores // 2)]
    all_group = [list(range(n_cores))]

    nc = bacc.Bacc("TRN2", target_bir_lowering=False, debug=False,
                   num_devices=n_cores)

    # ---- external inputs ----
    # h0T = (emb[tokens[own rows]] + pe[own rows]).T, staged on the host as
    # part of sharding (row gather + layout, no matmul compute)
    h0T = nc.dram_tensor("h0T", [ND, P, SO], BF16, kind="ExternalInput")
    # head-block masks, [r, t_row, q_col] (per-core data, see make_in_maps)
    maskT = nc.dram_tensor("maskT", [2, P, P], BF16, kind="ExternalInput")
    wts = {}
    for l in (1, 2):
        for nm in ("wk", "wv", "wo", "w1", "w2"):
            shp = [D, H] if nm == "w1" else ([H, D] if nm == "w2" else [D, D])
            wts[l, nm] = nc.dram_tensor(f"l{l}_{nm}", shp, BF16, kind="ExternalInput")
    w_out = nc.dram_tensor("w_out", [D, VS], BF16, kind="ExternalInput")
    logits = nc.dram_tensor("logits", [4, VS], F32, kind="ExternalOutput")
    dbg = None
    if stage != "full":
        dbg = nc.dram_tensor("dbg", [P, ND, S], BF16, kind="ExternalOutput")

    with tile.TileContext(nc) as tc:
        with (
            tc.tile_pool(name="sb", bufs=1) as sb,       # all SBUF, per-tag bufs
            tc.tile_pool(name="ps", bufs=1, space="PSUM") as ps_p,
            tc.tile_pool(name="dram", bufs=2, space="DRAM") as dram_p,
        ):
            # ---- constants ----
            mask_sb = sb.tile([P, 2, P], BF16, tag="mask")
            nc.sync.dma_start(mask_sb[:], maskT[:].rearrange("r t q -> t r q"))
            ones_sb = sb.tile([P, 1], BF16, tag="ones")
            nc.vector.memset(ones_sb[:], 1.0)
            ident = sb.tile([P, P], BF16, tag="ident")
            make_identity(nc, ident[:])

            # ---- h0 (pre-transposed on host) ----
            own_hT = sb.tile([P, ND, SO], BF16, tag="own", bufs=2)
            for i in range(ND):
                eng = nc.sync if i % 2 == 0 else nc.scalar
                eng.dma_start(own_hT[:, i, :], h0T[i])

            if stage == "h0":
                nc.sync.dma_start(dbg[:, :, :SO], own_hT[:])

            wk_sb = sb.tile([P, ND, D], BF16, tag="wkv", bufs=2)
            nc.sync.dma_start(wk_sb[:], wts[1, "wk"][:].rearrange("(k p) n -> p k n", p=P))
            wv_sb = sb.tile([P, ND, D], BF16, tag="wkv", bufs=2)
            nc.sync.dma_start(wv_sb[:], wts[1, "wv"][:].rearrange("(k p) n -> p k n", p=P))
            wo_sb = sb.tile([P, ND, D], BF16, tag="wo")
            nc.scalar.dma_start(wo_sb[:], wts[1, "wo"][:].rearrange("(k p) n -> p k n", p=P))

            cc_l_out = None
            blocks = {"h0": (), "kv": (1,), "att": (1,), "block1": (1,)}.get(stage, (1, 2))
            for l in blocks:
                # ============ K^T / V compute + pair-wise exchange ============
                cc_in_k = [dram_p.tile([D, KH], BF16, tag=f"cck{h}", name=f"cck{h}_{l}") for h in range(2)]
                cc_out_k = [dram_p.tile([2, D, KH], BF16, tag=f"ccko{h}", name=f"ccko{h}_{l}") for h in range(2)]
                cc_in_v = [dram_p.tile([MH * P, D], BF16, tag=f"ccv{h}", name=f"ccv{h}_{l}") for h in range(2)]
                cc_out_v = [dram_p.tile([2, MH * P, D], BF16, tag=f"ccvo{h}", name=f"ccvo{h}_{l}") for h in range(2)]

                for hh in range(2):
                    # kT own columns [hh*KH, (hh+1)*KH]
                    for i in range(ND):
                        pk = ps_p.tile([P, 512], F32, tag="mm", bufs=2, name=f"pk{l}{hh}{i}")
                        for k in range(ND):
                            nc.tensor.matmul(
                                pk[:], wk_sb[:, k, i * P : (i + 1) * P],
                                own_hT[:, k, hh * KH : (hh + 1) * KH],
                                start=(k == 0), stop=(k == ND - 1),
                            )
                        st = sb.tile([P, 512], BF16, tag="st", bufs=2, name=f"stk{l}{hh}{i}")
                        nc.vector.tensor_copy(st[:], pk[:])
                        eng = nc.sync if i % 2 == 0 else nc.scalar
                        eng.dma_start(cc_in_k[hh][i * P : (i + 1) * P, :], st[:])
                    nc.gpsimd.collective_compute(
                        "AllGather", mybir.AluOpType.bypass,
                        replica_groups=pair_groups,
                        ins=[cc_in_k[hh][:].opt()], outs=[cc_out_k[hh][:].opt()],
                    )
                    # v own row-tiles [hh*MH, (hh+1)*MH]
                    for m0 in range(MH):
                        m = hh * MH + m0
                        for ci in range(2):
                            pv = ps_p.tile([P, 512], F32, tag="mm", bufs=2, name=f"pv{l}{m}{ci}")
                            for k in range(ND):
                                nc.tensor.matmul(
                                    pv[:], own_hT[:, k, m * P : (m + 1) * P],
                                    wv_sb[:, k, ci * 512 : (ci + 1) * 512],
                                    start=(k == 0), stop=(k == ND - 1),
                                )
                            st = sb.tile([P, 512], BF16, tag="st", bufs=2, name=f"stv{l}{m}{ci}")
                            nc.vector.tensor_copy(st[:], pv[:])
                            eng = nc.sync if ci % 2 == 0 else nc.scalar
                            eng.dma_start(
                                cc_in_v[hh][m0 * P : (m0 + 1) * P, ci * 512 : (ci + 1) * 512],
                                st[:],
                            )
                    nc.gpsimd.collective_compute(
                        "AllGather", mybir.AluOpType.bypass,
                        replica_groups=pair_groups,
                        ins=[cc_in_v[hh][:].opt()], outs=[cc_out_v[hh][:].opt()],
                    )

                # v_all: stored slot s = r*NJ + m holds v rows of global tile
                # 2m + r; filled straight from the AllGather outputs
                v_all = sb.tile([P, NS, D], BF16, tag="vall", name=f"vall{l}")
                for hh in range(2):
                    for r in range(2):
                        eng = nc.sync if r == 0 else nc.scalar
                        eng.dma_start(
                            v_all[:, r * NJ + hh * MH : r * NJ + (hh + 1) * MH, :],
                            cc_out_v[hh][r].rearrange("(m p) d -> p m d", p=P),
                        )

                if stage == "kv":
                    for s in range(NS):
                        r, m = s // NJ, s % NJ
                        nc.sync.dma_start(
                            dbg[:, :, s * P : (s + 1) * P],
                            cc_out_k[m // MH][r, :, (m % MH) * P : (m % MH + 1) * P]
                            .rearrange("(i p) t -> p i t", p=P),
                        )
                    break

                # ============ attention: scoresT / max-free softmax ============
                # stored slots in AllGather arrival order: m 0..3 (both ranks)
                # land with half 0, m 4..7 with half 1
                slot_order = [r * NJ + m for m in range(MH) for r in range(2)] + \
                             [r * NJ + m for m in range(MH, NJ) for r in range(2)]
                expT = sb.tile([P, NS, SO], BF16, tag="expT", name=f"expT{l}")
                l_ps = [ps_p.tile([1, 512], F32, tag="lps", bufs=2, name=f"lps{l}{c}")
                        for c in range(2)]

                for si, s in enumerate(slot_order):
                    r, m = s // NJ, s % NJ
                    kTs = sb.tile([P, ND, P], BF16, tag="kts", bufs=2, name=f"kts{l}{s}")
                    eng = nc.sync if si % 2 == 0 else nc.scalar
                    eng.dma_start(
                        kTs[:],
                        cc_out_k[m // MH][r, :, (m % MH) * P : (m % MH + 1) * P]
                        .rearrange("(i p) t -> p i t", p=P),
                    )
                    # units: (q-chunk c, in-chunk col offset of the suffix head)
                    units = [(m // MH, (m % MH) * P)]
                    if m // MH == 0:
                        units.append((1, 0))
                    for (c, off) in units:
                        sc = ps_p.tile([P, 512], F32, tag="sc", bufs=2, name=f"sc{l}{s}{c}")
                        for k in range(ND):
                            nc.tensor.matmul(
                                sc[:, off:], kTs[:, k, :],
                                own_hT[:, k, c * 512 + off : (c + 1) * 512],
                                start=(k == 0), stop=(k == ND - 1),
                            )
                        if c == m // MH:
                            # suffix head block: causal mask (data per core)
                            nc.vector.tensor_add(
                                sc[:, off : off + P], sc[:, off : off + P],
                                mask_sb[:, r, :],
                            )
                        nc.scalar.activation(
                            expT[:, s, c * 512 + off : (c + 1) * 512], sc[:, off:],
                            mybir.ActivationFunctionType.Exp,
                        )
                        # softmax denominator: l[q] += sum_t exp[t, q]
                        nc.tensor.matmul(
                            l_ps[c][:, off:], ones_sb[:],
                            expT[:, s, c * 512 + off : (c + 1) * 512],
                            start=(si == 0), stop=(si == len(slot_order) - 1),
                            skip_group_check=True,
                        )

                # inv_l, broadcast to all partitions (GPSIMD; PE stays on matmuls)
                inv_bc = []
                for c in range(2):
                    inv_sb = sb.tile([1, 512], F32, tag="inv", bufs=1, name=f"inv{l}{c}")
                    nc.vector.reciprocal(inv_sb[:], l_ps[c][:])
                    bc = sb.tile([P, 512], F32, tag="invbc", bufs=2, name=f"invbc{l}{c}")
                    nc.gpsimd.partition_broadcast(bc[:], inv_sb[:])
                    inv_bc.append(bc)

                # attn @ v -> h_attnT [d-block, q], slot-accumulated in PSUM;
                # the PSUM->SBUF drain applies the 1/l normalization
                h_attnT = sb.tile([P, ND, SO], BF16, tag="hat", name=f"hat{l}")
                for c in range(2):
                    slots_c = [s for s in slot_order if (s % NJ) // MH <= c]
                    for i in range(ND):
                        av = ps_p.tile([P, 512], F32, tag="av", bufs=2, name=f"av{l}{c}{i}")
                        for sj, s in enumerate(slots_c):
                            m = s % NJ
                            off = (m % MH) * P if (m // MH) == c else 0
                            nc.tensor.matmul(
                                av[:, off:],
                                v_all[:, s, i * P : (i + 1) * P],
                                expT[:, s, c * 512 + off : (c + 1) * 512],
                                start=(sj == 0), stop=(sj == len(slots_c) - 1),
                                skip_group_check=True,
                            )
                        nc.vector.tensor_mul(
                            h_attnT[:, i, c * 512 : (c + 1) * 512], av[:], inv_bc[c][:],
                        )

                if stage == "att":
                    nc.sync.dma_start(dbg[:, :, :SO], h_attnT[:])
                    break

                # ============ Wo + residual -> h_resT ============
                h_resT = sb.tile([P, ND, SO], BF16, tag="hres", name=f"hres{l}")
                for i in range(ND):
                    for c in range(2):
                        po = ps_p.tile([P, 512], F32, tag="mm", bufs=2, name=f"po{l}{i}{c}")
                        for k in range(ND):
                            nc.tensor.matmul(
                                po[:], wo_sb[:, k, i * P : (i + 1) * P],
                                h_attnT[:, k, c * 512 : (c + 1) * 512],
                                start=(k == 0), stop=(k == ND - 1),
                            )
                        nc.vector.tensor_add(
                            h_resT[:, i, c * 512 : (c + 1) * 512], po[:],
                            own_hT[:, i, c * 512 : (c + 1) * 512],
                        )

                if l == 1 and stage == "full":
                    # prefetch block-2 wo during block-1 FFN
                    wo_sb = sb.tile([P, ND, D], BF16, tag="wo", name="wo2")
                    nc.scalar.dma_start(wo_sb[:], wts[2, "wo"][:].rearrange("(k p) n -> p k n", p=P))

                # ============ FFN (split q-chunks; block2 runs the half with
                # the final token first so the logits AllGather overlaps) ====
                own_next = sb.tile([P, ND, SO], BF16, tag="own", bufs=2, name=f"own{l}")
                if l == 2:
                    last_col = sb.tile([P, ND], BF16, tag="lastcol")
                chunk_order = (0, 1) if l == 1 else (1, 0)
                for ci, c in enumerate(chunk_order):
                    qoff = c * 512
                    midT = sb.tile([P, NH, 512], BF16, tag="vall", name=f"midT{l}{c}")
                    for ch in range(NH // W1CH):
                        w1_sb = sb.tile([P, ND, W1CH * P], BF16, tag="wkv", bufs=2,
                                        name=f"w1_{l}{c}{ch}")
                        nc.sync.dma_start(
                            w1_sb[:],
                            wts[l, "w1"][:, ch * W1CH * P : (ch + 1) * W1CH * P]
                            .rearrange("(k p) n -> p k n", p=P),
                        )
                        for hb in range(W1CH):
                            g = ch * W1CH + hb
                            pm = ps_p.tile([P, 512], F32, tag="mm", bufs=2,
                                           name=f"pm{l}{c}{g}")
                            for k in range(ND):
                                nc.tensor.matmul(
                                    pm[:], w1_sb[:, k, hb * P : (hb + 1) * P],
                                    h_resT[:, k, qoff : qoff + 512],
                                    start=(k == 0), stop=(k == ND - 1),
                                )
                            nc.vector.tensor_scalar_max(midT[:, g, :], pm[:], 0.0)

                    if l == 1 and ci == 1 and stage == "full":
                        # prefetch block-2 wk/wv into the freed wkv slots while
                        # the w2 phase runs (slots are done streaming w1)
                        wk_sb = sb.tile([P, ND, D], BF16, tag="wkv", bufs=2, name="wk2")
                        nc.sync.dma_start(wk_sb[:], wts[2, "wk"][:].rearrange("(k p) n -> p k n", p=P))
                        wv_sb = sb.tile([P, ND, D], BF16, tag="wkv", bufs=2, name="wv2")
                        nc.sync.dma_start(wv_sb[:], wts[2, "wv"][:].rearrange("(k p) n -> p k n", p=P))

                    if l == 2 and ci == 1:
                        # logits: lhsT prep + first half of the vocab chunks,
                        # interleaved between the w1 and w2 phases of the last
                        # FFN chunk (the 8-core AllGather is long done by now)
                        h_last = sb.tile([4, ND, P], BF16, tag="hlast")
                        nc.sync.dma_start(
                            h_last[:],
                            cc_l_out[:].rearrange("r (i p) -> r i p", p=P)[1::2],
                        )
                        lhsT = sb.tile([P, ND, 4], BF16, tag="lhsT")
                        for i in range(ND):
                            ps_t = ps_p.tile([P, 512], BF16, tag="sc", bufs=2, name=f"pst{i}")
                            nc.tensor.transpose(ps_t[:, :4], h_last[:, i, :], ident[:4, :4])
                            nc.vector.tensor_copy(lhsT[:, i, :], ps_t[:, :4])

                        def logits_chunks(lo, hi, wo_stream):
                            for vc in range(lo, hi):
                                pl = ps_p.tile([P, 512], F32, tag="av", bufs=2, name=f"pl{vc}")
                                for k in range(ND):
                                    nc.tensor.matmul(
                                        pl[:4, :VC], lhsT[:, k, :],
                                        wo_stream[:, k, (vc - lo) * VC : (vc - lo + 1) * VC],
                                        start=(k == 0), stop=(k == ND - 1),
                                    )
                                lg = sb.tile([4, VC], F32, tag="lg", bufs=2, name=f"lg{vc}")
                                nc.vector.tensor_copy(lg[:], pl[:4, :VC])
                                eng = nc.sync if vc % 2 == 0 else nc.scalar
                                eng.dma_start(logits[:, vc * VC : (vc + 1) * VC], lg[:])

                        wo_ta = sb.tile([P, ND, 4 * VC], BF16, tag="expT", name="wo_ta")
                        nc.sync.dma_start(
                            wo_ta[:],
                            w_out[:, : 4 * VC].rearrange("(k p) n -> p k n", p=P),
                        )
                        logits_chunks(0, 4, wo_ta)
                        wo_tb = sb.tile([P, ND, 4 * VC], BF16, tag="expT", name="wo_tb")
                        nc.scalar.dma_start(
                            wo_tb[:],
                            w_out[:, 4 * VC :].rearrange("(k p) n -> p k n", p=P),
                        )

                    for i in range(ND):
                        w2_sb = sb.tile([P, NH, P], BF16, tag="kts", bufs=2,
                                        name=f"w2_{l}{c}{i}")
                        nc.scalar.dma_start(
                            w2_sb[:],
                            wts[l, "w2"][:, i * P : (i + 1) * P]
                            .rearrange("(k p) n -> p k n", p=P),
                        )
                        pw = ps_p.tile([P, 512], F32, tag="mm", bufs=2, name=f"pw{l}{c}{i}")
                        for hb in range(NH):
                            nc.tensor.matmul(
                                pw[:], w2_sb[:, hb, :], midT[:, hb, :],
                                start=(hb == 0), stop=(hb == NH - 1),
                            )
                        nc.vector.tensor_add(
                            own_next[:, i, qoff : qoff + 512], pw[:],
                            h_resT[:, i, qoff : qoff + 512],
                        )
                        if l == 2 and c == 1:
                            nc.vector.tensor_add(
                                last_col[:, i : i + 1], pw[:, 511:512],
                                h_resT[:, i, SO - 1 : SO],
                            )
                    if l == 2 and c == 1:
                        # logits prologue: transpose last col, 8-core AllGather
                        # (overlaps the remaining chunk-lo FFN work)
                        lc_t = sb.tile([ND, P], BF16, tag="lct")
                        ps_lc = ps_p.tile([P, P], BF16, tag="sc", bufs=2, name="pslc")
                        nc.tensor.transpose(ps_lc[:ND, :P], last_col[:], ident[:])
                        nc.vector.tensor_copy(lc_t[:], ps_lc[:ND, :P])
                        cc_l_in = dram_p.tile([D], BF16, tag="ccl")
                        cc_l_out = dram_p.tile([n_cores, D], BF16, tag="cclo")
                        nc.sync.dma_start(cc_l_in[:].rearrange("(i p) -> i p", p=P), lc_t[:])
                        nc.gpsimd.collective_compute(
                            "AllGather", mybir.AluOpType.bypass,
                            replica_groups=all_group,
                            ins=[cc_l_in[:].opt()], outs=[cc_l_out[:].opt()],
                        )

                own_hT = own_next
                if stage == "block1":
                    nc.sync.dma_start(dbg[:, :, :SO], own_hT[:])
                    break

            if stage == "full":
                # remaining logits chunks (most of the stream overlapped FFN)
                logits_chunks(4, NVC, wo_tb)

    nc.compile()
    return nc


# ----------------------------------------------------------------------------
# host side
# ----------------------------------------------------------------------------

def make_in_maps(tokens, emb, pe, weights, S=2048, D=1024, H=4096, V=32000,
                 n_cores=8):
    """weights: dict with l{1,2}_{wk,wv,wo,w1,w2} and w_out (fp32 numpy)."""
    bf = ml_dtypes.bfloat16
    NJ = (S // P) // 2
    ND = D // P
    SO = S // 2
    VS = V // n_cores
    emb_f = np.ascontiguousarray(emb, dtype=np.float32)
    pe_f = np.asarray(pe, dtype=np.float32)
    scale = 1.0 / np.sqrt(float(D))
    w_bf = {}
    for l in (1, 2):
        w_bf[f"l{l}_wk"] = (np.asarray(weights[f"l{l}_wk"], np.float32) * scale).astype(bf)
        for nm in ("wv", "wo", "w1", "w2"):
            w_bf[f"l{l}_{nm}"] = np.asarray(weights[f"l{l}_{nm}"], np.float32).astype(bf)
    w_out_bf = np.asarray(weights["w_out"], np.float32).astype(bf)

    tokens = np.asarray(tokens)
    in_maps = []
    # scoresT head-block masks: [t_row, q_col] within the diagonal 128-tile;
    # allowed iff q >= t  ->  NEG on the strict lower triangle
    tri = np.tril(np.full((P, P), NEG, np.float32), k=-1)
    for c in range(n_cores):
        b, t = c // 2, c % 2
        own_rows = np.concatenate(
            [np.arange((2 * j + t) * P, (2 * j + t + 1) * P) for j in range(NJ)]
        )
        h0 = (emb_f[tokens[b, own_rows]] + pe_f[own_rows]).astype(np.float32)
        h0T = np.ascontiguousarray(h0.T).reshape(ND, P, SO).astype(bf)
        # head-block mask per stored rank r: global q-tile 2m+t vs key tile
        # 2m+r: t==r -> diagonal triangle; t<r -> fully masked; t>r -> allowed
        maskT = np.zeros((2, P, P), np.float32)
        for r in range(2):
            if t == r:
                maskT[r] = tri
            elif t < r:
                maskT[r] = NEG
        in_map = {
            "h0T": h0T,
            "maskT": maskT.astype(bf),
            "w_out": np.ascontiguousarray(w_out_bf[:, c * VS : (c + 1) * VS]),
        }
        in_map.update(w_bf)
        in_maps.append(in_map)
    return in_maps


_NC_CACHE = {}


def _get_nc(key=(2048, 1024, 4096, 32000, 8)):
    if key not in _NC_CACHE:
        _NC_CACHE[key] = build_nc(*key)
    return _NC_CACHE[key]


def kernel(tokens, emb, pe, l1_wk, l1_wv, l1_wo, l1_w1, l1_w2,
           l2_wk, l2_wv, l2_wo, l2_w1, l2_w2, w_out):
    S = int(np.asarray(tokens).shape[1])
    D = int(np.asarray(emb).shape[1])
    H = int(np.asarray(l1_w1).shape[1])
    V = int(np.asarray(emb).shape[0])
    n_cores = 8
    nc = _get_nc((S, D, H, V, n_cores))
    weights = dict(
        l1_wk=l1_wk, l1_wv=l1_wv, l1_wo=l1_wo, l1_w1=l1_w1, l1_w2=l1_w2,
        l2_wk=l2_wk, l2_wv=l2_wv, l2_wo=l2_wo, l2_w1=l2_w1, l2_w2=l2_w2,
        w_out=w_out,
    )
    in_maps = make_in_maps(tokens, emb, pe, weights, S, D, H, V, n_cores)
    try:
        res = run_bass_kernel_spmd(nc, in_maps, core_ids=list(range(n_cores)))
    except Exception:
        # a previous crashed run can leave the device wedged; one retry
        # (fresh NRT session) clears it
        import os
        os.environ.setdefault("NEURON_RT_RESET_CORES", "1")
        res = run_bass_kernel_spmd(nc, in_maps, core_ids=list(range(n_cores)))
    VS = V // n_cores
    out = np.zeros((np.asarray(tokens).shape[0], V), np.float32)
    for c in range(n_cores):
        out[:, c * VS : (c + 1) * VS] = res.results[c]["logits"]
    return out


# revision 13
# speedup vs baseline: 1.1054x; 1.0607x over previous
"""Bass/Trainium2 kernel for a 2-block single-head causal transformer.

Strategy (8 NeuronCores): data-parallel over batch (B=4 -> 4 core pairs),
sequence-parallel within each pair. Each core owns the interleaved global
query tiles {2j + t} (t = core parity), so the instruction stream is
identical on every core; all per-core variation (embedded rows, causal
masks, vocab slice) is input data.

v2 design (vs the first working version):
- h0 arrives from the host already transposed -> zero PE transposes at start.
- Attention computes TRANSPOSED scores  scoresT[t, q] = k_t . q_q  so the
  post-softmax attn@v contraction (over t) needs NO transposes at all, and
  softmax runs max-free (scores are bounded ~20, exp is fp32-safe), removing
  the cross-range reduce_max. The softmax denominator l[q] is recovered with
  ones-vector matmuls on the PE and folded back in via a GPSIMD
  partition_broadcast + the (mandatory anyway) PSUM->SBUF drain multiply.
- K^T / V are exchanged pair-wise in column/row halves with AllGathers
  interleaved (k0, v0, k1, v1); attention streams over stored key tiles in
  AllGather-arrival order, so collective latency hides under score compute.
- Stored-slot indexing is rank-major (slot s = r*8 + m), which makes the
  instruction stream core-invariant; the one-block causal asymmetry between
  the two parities is absorbed by per-core mask data (head-block masks).
- Block-2's first K/V half (compute + AllGathers) is emitted in the middle
  of block-1's FFN so the collectives complete before block-2 attention.
- The scalar engine runs ONLY the softmax exp; all DMA issue lives on the
  sync/gpsimd queues (and vector/scalar once, for the initial weight load).
- The final-token logits machinery (w_out streaming, logits matmuls) is
  interleaved into the block-2 FFN so only a small drain remains at the end.

Everything is bf16 into the PE array with fp32 PSUM accumulation.
"""

import sys

sys.path.insert(0, "/opt/trn_rl_repo")

import numpy as np
import ml_dtypes

import concourse.bass as bass
import concourse.mybir as mybir
import concourse.tile as tile
from concourse import bacc
from concourse.bass_utils import run_bass_kernel_spmd
from concourse.masks import make_identity

BF16 = mybir.dt.bfloat16
F32 = mybir.dt.float32
P = 128
NEG = -30000.0


def build_nc(S=2048, D=1024, H=4096, V=32000, n_cores=8, stage="full"):
    """Build the SPMD Bass program (identical on all cores).

    stage: "h0" | "kv" | "att" | "block1" | "full" — truncate after the named
    phase and dump an intermediate to `dbg` (debug).
    """
    NJ = (S // P) // 2          # own q-tiles per core
    ND = D // P                 # d blocks
    NH = H // P                 # h blocks
    SO = S // 2                 # own rows per core
    NS = 2 * NJ                 # stored key tiles (both ranks), s = r*NJ + m
    VS = V // n_cores           # vocab slice per core
    KH = SO // 2                # kT columns per AllGather half
    MH = NJ // 2                # v row-tiles per AllGather half
    W1CH = 8                    # h-blocks per streamed w1 chunk
    VC = 500                    # logits n-chunk
    NVC = VS // VC              # logits chunks (8)
    pair_groups = [[2 * i, 2 * i + 1] for i in range(n_cores // 2)]
    all_group = [list(range(n_cores))]

    nc = bacc.Bacc("TRN2", target_bir_lowering=False, debug=False,
                   num_devices=n_cores)

    # ---- external inputs ----
    # h0T = (emb[tokens[own rows]] + pe[own rows]).T, staged on the host as
    # part of sharding (row gather + layout, no matmul compute)
    h0T = nc.dram_tensor("h0T", [ND, P, SO], BF16, kind="ExternalInput")
    # head-block masks, [r, t_row, q_col] (per-core data, see make_in_maps)
    maskT = nc.dram_tensor("maskT", [2, P, P], BF16, kind="ExternalInput")
    wts = {}
    for l in (1, 2):
        for nm in ("wk", "wv", "wo", "w1", "w2"):
            shp = [D, H] if nm == "w1" else ([H, D] if nm == "w2" else [D, D])
            wts[l, nm] = nc.dram_tensor(f"l{l}_{nm}", shp, BF16, kind="ExternalInput")
    w_out = nc.dram_tensor("w_out", [D, VS], BF16, kind="ExternalInput")
    logits = nc.dram_tensor("logits", [4, VS], F32, kind="ExternalOutput")
    dbg = None
    if stage != "full":
        dbg = nc.dram_tensor("dbg", [P, ND, S], BF16, kind="ExternalOutput")

    with tile.TileContext(nc) as tc:
        with (
            tc.tile_pool(name="sb", bufs=1) as sb,       # all SBUF, per-tag bufs
            tc.tile_pool(name="ps", bufs=1, space="PSUM") as ps_p,
            tc.tile_pool(name="dram", bufs=2, space="DRAM") as dram_p,
        ):
            # ---- constants ----
            mask_sb = sb.tile([P, 2, P], BF16, tag="mask")
            nc.sync.dma_start(mask_sb[:], maskT[:].rearrange("r t q -> t r q"))
            ones_sb = sb.tile([P, 1], BF16, tag="ones")
            nc.vector.memset(ones_sb[:], 1.0)
            ident = sb.tile([P, P], BF16, tag="ident")
            make_identity(nc, ident[:])

            # ---- initial loads: h0T + wk + wv interleaved over 4 queues so
            # the first K^T matmul group's operands land fast ----
            q4 = [nc.sync, nc.gpsimd, nc.scalar]
            own_hT = sb.tile([P, ND, SO], BF16, tag="own", bufs=2)
            wk_sb = sb.tile([P, ND, D], BF16, tag="wkv", bufs=2)
            wv_sb = sb.tile([P, ND, D], BF16, tag="wkv", bufs=2)
            for k in range(ND):
                q4[k % 3].dma_start(wk_sb[:, k, :], wts[1, "wk"][k * P : (k + 1) * P, :])
                q4[(k + 1) % 3].dma_start(own_hT[:, k, :], h0T[k])
            for k in range(ND):
                q4[(k + 2) % 3].dma_start(wv_sb[:, k, :], wts[1, "wv"][k * P : (k + 1) * P, :])
            wo_sb = sb.tile([P, ND, D], BF16, tag="wo")
            nc.sync.dma_start(wo_sb[:], wts[1, "wo"][:].rearrange("(k p) n -> p k n", p=P))

            if stage == "h0":
                nc.sync.dma_start(dbg[:, :, :SO], own_hT[:])

            st_ctr = [0]

            def stage_out(ps_ap, dst_ap):
                st = sb.tile([P, 512], BF16, tag="st", bufs=4, name=f"st{st_ctr[0]}")
                st_ctr[0] += 1
                nc.vector.tensor_copy(st[:], ps_ap)
                eng = nc.sync if st_ctr[0] % 2 == 0 else nc.gpsimd
                eng.dma_start(dst_ap, st[:])

            # per-block state
            S_ = {}

            def emit_kv_half(l, hh, hT, wk_ap, wv_ap):
                """K^T / V for own half hh + AllGathers + v_all half fill."""
                if hh == 0:
                    S_[l, "cc_in_k"] = [dram_p.tile([D, KH], BF16, tag=f"cck{h}", name=f"cck{h}_{l}") for h in range(2)]
                    S_[l, "cc_out_k"] = [dram_p.tile([2, D, KH], BF16, tag=f"ccko{h}", name=f"ccko{h}_{l}") for h in range(2)]
                    S_[l, "cc_in_v"] = [dram_p.tile([MH * P, D], BF16, tag=f"ccv{h}", name=f"ccv{h}_{l}") for h in range(2)]
                    S_[l, "cc_out_v"] = [dram_p.tile([2, MH * P, D], BF16, tag=f"ccvo{h}", name=f"ccvo{h}_{l}") for h in range(2)]
                for i in range(ND):
                    pk = ps_p.tile([P, 512], F32, tag="mm", bufs=2, name=f"pk{l}{hh}{i}")
                    for k in range(ND):
                        nc.tensor.matmul(
                            pk[:], wk_ap[:, k, i * P : (i + 1) * P],
                            hT[:, k, hh * KH : (hh + 1) * KH],
                            start=(k == 0), stop=(k == ND - 1),
                        )
                    stage_out(pk[:], S_[l, "cc_in_k"][hh][i * P : (i + 1) * P, :])
                nc.gpsimd.collective_compute(
                    "AllGather", mybir.AluOpType.bypass,
                    replica_groups=pair_groups,
                    ins=[S_[l, "cc_in_k"][hh][:].opt()],
                    outs=[S_[l, "cc_out_k"][hh][:].opt()],
                )
                for m0 in range(MH):
                    m = hh * MH + m0
                    for ci in range(2):
                        pv = ps_p.tile([P, 512], F32, tag="mm", bufs=2, name=f"pv{l}{m}{ci}")
                        for k in range(ND):
                            nc.tensor.matmul(
                                pv[:], hT[:, k, m * P : (m + 1) * P],
                                wv_ap[:, k, ci * 512 : (ci + 1) * 512],
                                start=(k == 0), stop=(k == ND - 1),
                            )
                        stage_out(pv[:], S_[l, "cc_in_v"][hh][m0 * P : (m0 + 1) * P, ci * 512 : (ci + 1) * 512])
                nc.gpsimd.collective_compute(
                    "AllGather", mybir.AluOpType.bypass,
                    replica_groups=pair_groups,
                    ins=[S_[l, "cc_in_v"][hh][:].opt()],
                    outs=[S_[l, "cc_out_v"][hh][:].opt()],
                )

            def emit_vall_fill(l, hh):
                # fill this half's v_all slots straight from the gather output
                if (l, "v_all") not in S_:
                    S_[l, "v_all"] = sb.tile([P, NS, D], BF16, tag="vall", name=f"vall{l}")
                for r in range(2):
                    eng = nc.sync if r == 0 else nc.gpsimd
                    eng.dma_start(
                        S_[l, "v_all"][:, r * NJ + hh * MH : r * NJ + (hh + 1) * MH, :],
                        S_[l, "cc_out_v"][hh][r].rearrange("(m p) d -> p m d", p=P),
                    )

            def emit_attention(l, hT):
                """scoresT -> max-free exp -> l -> attn@v -> h_attnT."""
                cc_out_k = S_[l, "cc_out_k"]
                v_all = S_[l, "v_all"]
                slot_order = [r * NJ + m for m in range(MH) for r in range(2)] + \
                             [r * NJ + m for m in range(MH, NJ) for r in range(2)]
                expT = sb.tile([P, NS, SO], BF16, tag="expT", name=f"expT{l}")
                l_ps = [ps_p.tile([1, 512], F32, tag="lps", bufs=2, name=f"lps{l}{c}")
                        for c in range(2)]

                for si, s in enumerate(slot_order):
                    r, m = s // NJ, s % NJ
                    kTs = sb.tile([P, ND, P], BF16, tag="kts", bufs=2, name=f"kts{l}{s}")
                    eng = nc.sync if si % 2 == 0 else nc.gpsimd
                    eng.dma_start(
                        kTs[:],
                        cc_out_k[m // MH][r, :, (m % MH) * P : (m % MH + 1) * P]
                        .rearrange("(i p) t -> p i t", p=P),
                    )
                    units = [(m // MH, (m % MH) * P)]
                    if m // MH == 0:
                        units.append((1, 0))
                    for (c, off) in units:
                        sc = ps_p.tile([P, 512], F32, tag="sc", bufs=2, name=f"sc{l}{s}{c}")
                        for k in range(ND):
                            nc.tensor.matmul(
                                sc[:, off:], kTs[:, k, :],
                                hT[:, k, c * 512 + off : (c + 1) * 512],
                                start=(k == 0), stop=(k == ND - 1),
                            )
                        if c == m // MH:
                            # suffix head block: causal mask (data per core)
                            nc.vector.tensor_add(
                                sc[:, off : off + P], sc[:, off : off + P],
                                mask_sb[:, r, :],
                            )
                        nc.scalar.activation(
                            expT[:, s, c * 512 + off : (c + 1) * 512], sc[:, off:],
                            mybir.ActivationFunctionType.Exp,
                        )
                        nc.tensor.matmul(
                            l_ps[c][:, off:], ones_sb[:],
                            expT[:, s, c * 512 + off : (c + 1) * 512],
                            start=(si == 0), stop=(si == len(slot_order) - 1),
                            skip_group_check=True,
                        )

                inv_bc = []
                for c in range(2):
                    inv_sb = sb.tile([1, 512], F32, tag="inv", bufs=1, name=f"inv{l}{c}")
                    nc.vector.reciprocal(inv_sb[:], l_ps[c][:])
                    bc = sb.tile([P, 512], F32, tag="invbc", bufs=2, name=f"invbc{l}{c}")
                    nc.gpsimd.partition_broadcast(bc[:], inv_sb[:])
                    inv_bc.append(bc)

                h_attnT = sb.tile([P, ND, SO], BF16, tag="hat", name=f"hat{l}")
                for c in range(2):
                    slots_c = [s for s in slot_order if (s % NJ) // MH <= c]
                    for i in range(ND):
                        av = ps_p.tile([P, 512], F32, tag="av", bufs=2, name=f"av{l}{c}{i}")
                        for sj, s in enumerate(slots_c):
                            m = s % NJ
                            off = (m % MH) * P if (m // MH) == c else 0
                            nc.tensor.matmul(
                                av[:, off:],
                                v_all[:, s, i * P : (i + 1) * P],
                                expT[:, s, c * 512 + off : (c + 1) * 512],
                                start=(sj == 0), stop=(sj == len(slots_c) - 1),
                                skip_group_check=True,
                            )
                        nc.vector.tensor_mul(
                            h_attnT[:, i, c * 512 : (c + 1) * 512], av[:], inv_bc[c][:],
                        )
                return h_attnT

            def emit_wo(l, hT, h_attnT, wo_ap):
                h_resT = sb.tile([P, ND, SO], BF16, tag="hres", name=f"hres{l}")
                for i in range(ND):
                    for c in range(2):
                        po = ps_p.tile([P, 512], F32, tag="mm", bufs=2, name=f"po{l}{i}{c}")
                        for k in range(ND):
                            nc.tensor.matmul(
                                po[:], wo_ap[:, k, i * P : (i + 1) * P],
                                h_attnT[:, k, c * 512 : (c + 1) * 512],
                                start=(k == 0), stop=(k == ND - 1),
                            )
                        nc.vector.tensor_add(
                            h_resT[:, i, c * 512 : (c + 1) * 512], po[:],
                            hT[:, i, c * 512 : (c + 1) * 512],
                        )
                return h_resT

            def emit_ffn_chunk(l, c, h_resT, own_next, last_col=None,
                               pre_w2=None, post_w1=None):
                qoff = c * 512
                midT = sb.tile([P, NH, 512], BF16, tag="vall", name=f"midT{l}{c}")
                for ch in range(NH // W1CH):
                    w1_sb = sb.tile([P, ND, W1CH * P], BF16, tag="wkv", bufs=2,
                                    name=f"w1_{l}{c}{ch}")
                    nc.sync.dma_start(
                        w1_sb[:],
                        wts[l, "w1"][:, ch * W1CH * P : (ch + 1) * W1CH * P]
                        .rearrange("(k p) n -> p k n", p=P),
                    )
                    for hb in range(W1CH):
                        g = ch * W1CH + hb
                        pm = ps_p.tile([P, 512], F32, tag="mm", bufs=2,
                                       name=f"pm{l}{c}{g}")
                        for k in range(ND):
                            nc.tensor.matmul(
                                pm[:], w1_sb[:, k, hb * P : (hb + 1) * P],
                                h_resT[:, k, qoff : qoff + 512],
                                start=(k == 0), stop=(k == ND - 1),
                            )
                        nc.vector.tensor_scalar_max(midT[:, g, :], pm[:], 0.0)
                if post_w1 is not None:
                    post_w1()
                if pre_w2 is not None:
                    pre_w2()
                for i in range(ND):
                    w2_sb = sb.tile([P, NH, P], BF16, tag="kts", bufs=2,
                                    name=f"w2_{l}{c}{i}")
                    nc.gpsimd.dma_start(
                        w2_sb[:],
                        wts[l, "w2"][:, i * P : (i + 1) * P]
                        .rearrange("(k p) n -> p k n", p=P),
                    )
                    pw = ps_p.tile([P, 512], F32, tag="mm", bufs=2, name=f"pw{l}{c}{i}")
                    for hb in range(NH):
                        nc.tensor.matmul(
                            pw[:], w2_sb[:, hb, :], midT[:, hb, :],
                            start=(hb == 0), stop=(hb == NH - 1),
                        )
                    nc.vector.tensor_add(
                        own_next[:, i, qoff : qoff + 512], pw[:],
                        h_resT[:, i, qoff : qoff + 512],
                    )
                    if last_col is not None:
                        nc.vector.tensor_add(
                            last_col[:, i : i + 1], pw[:, 511:512],
                            h_resT[:, i, SO - 1 : SO],
                        )

            # ================= block 1 =================
            emit_kv_half(1, 0, own_hT, wk_sb, wv_sb)
            emit_vall_fill(1, 0)
            emit_kv_half(1, 1, own_hT, wk_sb, wv_sb)
            emit_vall_fill(1, 1)
            if stage == "kv":
                for s in range(NS):
                    r, m = s // NJ, s % NJ
                    nc.sync.dma_start(
                        dbg[:, :, s * P : (s + 1) * P],
                        S_[1, "cc_out_k"][m // MH][r, :, (m % MH) * P : (m % MH + 1) * P]
                        .rearrange("(i p) t -> p i t", p=P),
                    )
            elif stage in ("att", "block1", "full"):
                h_attnT = emit_attention(1, own_hT)
                if stage == "att":
                    nc.sync.dma_start(dbg[:, :, :SO], h_attnT[:])
                else:
                    h_resT = emit_wo(1, own_hT, h_attnT, wo_sb)
                    # prefetch block-2 wo while the FFN runs
                    wo2_sb = sb.tile([P, ND, D], BF16, tag="wo", name="wo2")
                    nc.sync.dma_start(wo2_sb[:], wts[2, "wo"][:].rearrange("(k p) n -> p k n", p=P))
                    own2 = sb.tile([P, ND, SO], BF16, tag="own", bufs=2, name="own2")

                    wkv2 = sb.tile([P, 2, ND, D], BF16, tag="expT", name="wkv2")

                    def prefetch_wkv2():
                        for k in range(ND):
                            eng = nc.sync if k % 2 == 0 else nc.gpsimd
                            eng.dma_start(wkv2[:, 0, k, :], wts[2, "wk"][k * P : (k + 1) * P, :])
                            eng.dma_start(wkv2[:, 1, k, :], wts[2, "wv"][k * P : (k + 1) * P, :])

                    emit_ffn_chunk(1, 0, h_resT, own2, pre_w2=prefetch_wkv2)
                    if stage == "full":
                        # block-2 K/V first half mid-FFN: its AllGathers run
                        # under block-1's remaining FFN work
                        emit_kv_half(2, 0, own2, wkv2[:, 0], wkv2[:, 1])
                    emit_ffn_chunk(1, 1, h_resT, own2)
                    if stage == "block1":
                        nc.sync.dma_start(dbg[:, :, :SO], own2[:])

            # ================= block 2 =================
            if stage == "full":
                emit_vall_fill(2, 0)
                emit_kv_half(2, 1, own2, wkv2[:, 0], wkv2[:, 1])
                emit_vall_fill(2, 1)
                h_attnT2 = emit_attention(2, own2)
                h_resT2 = emit_wo(2, own2, h_attnT2, wo2_sb)
                own3 = sb.tile([P, ND, SO], BF16, tag="own", bufs=2, name="own3")
                last_col = sb.tile([P, ND], BF16, tag="lastcol")

                # FFN half with the final token first; AllGather of the last
                # token's activations overlaps the other half
                emit_ffn_chunk(2, 1, h_resT2, own3, last_col=last_col)
                lc_t = sb.tile([ND, P], BF16, tag="lct")
                ps_lc = ps_p.tile([P, P], BF16, tag="sc", bufs=2, name="pslc")
                nc.tensor.transpose(ps_lc[:ND, :P], last_col[:], ident[:])
                nc.vector.tensor_copy(lc_t[:], ps_lc[:ND, :P])
                cc_l_in = dram_p.tile([D], BF16, tag="ccl")
                cc_l_out = dram_p.tile([n_cores, D], BF16, tag="cclo")
                nc.sync.dma_start(cc_l_in[:].rearrange("(i p) -> i p", p=P), lc_t[:])
                nc.gpsimd.collective_compute(
                    "AllGather", mybir.AluOpType.bypass,
                    replica_groups=all_group,
                    ins=[cc_l_in[:].opt()], outs=[cc_l_out[:].opt()],
                )

                lhsT = sb.tile([P, ND, 4], BF16, tag="lhsT")
                lg_ctr = [0]

                def logits_chunks(lo, hi, wo_stream):
                    for vc in range(lo, hi):
                        pl = ps_p.tile([P, 512], F32, tag="av", bufs=2, name=f"pl{vc}")
                        for k in range(ND):
                            nc.tensor.matmul(
                                pl[:4, :VC], lhsT[:, k, :],
                                wo_stream[:, k, (vc - lo) * VC : (vc - lo + 1) * VC],
                                start=(k == 0), stop=(k == ND - 1),
                            )
                        lg = sb.tile([4, VC], F32, tag="lg", bufs=1, name=f"lg{vc}")
                        nc.vector.tensor_copy(lg[:], pl[:4, :VC])
                        eng = nc.sync if vc % 2 == 0 else nc.gpsimd
                        eng.dma_start(logits[:, vc * VC : (vc + 1) * VC], lg[:])

                wo_ta = sb.tile([P, ND, 4 * VC], BF16, tag="expT", name="wo_ta")
                wo_tb = None

                def logits_mid():
                    # lhsT prep + first half of the vocab chunks, interleaved
                    # between w1 and w2 of the last FFN chunk (the 8-core
                    # AllGather is long done by now)
                    nonlocal wo_tb
                    h_last = sb.tile([4, ND, P], BF16, tag="hlast")
                    nc.sync.dma_start(
                        h_last[:],
                        cc_l_out[:].rearrange("r (i p) -> r i p", p=P)[1::2],
                    )
                    for i in range(ND):
                        ps_t = ps_p.tile([P, 512], BF16, tag="sc", bufs=2, name=f"pst{i}")
                        nc.tensor.transpose(ps_t[:, :4], h_last[:, i, :], ident[:4, :4])
                        nc.vector.tensor_copy(lhsT[:, i, :], ps_t[:, :4])
                    nc.sync.dma_start(
                        wo_ta[:],
                        w_out[:, : 4 * VC].rearrange("(k p) n -> p k n", p=P),
                    )
                    logits_chunks(0, 4, wo_ta)
                    wo_tb = sb.tile([P, ND, 4 * VC], BF16, tag="expT", name="wo_tb")
                    nc.gpsimd.dma_start(
                        wo_tb[:],
                        w_out[:, 4 * VC :].rearrange("(k p) n -> p k n", p=P),
                    )

                emit_ffn_chunk(2, 0, h_resT2, own3, post_w1=logits_mid)
                logits_chunks(4, NVC, wo_tb)

    nc.compile()
    return nc


# ----------------------------------------------------------------------------
# host side
# ----------------------------------------------------------------------------

def make_in_maps(tokens, emb, pe, weights, S=2048, D=1024, H=4096, V=32000,
                 n_cores=8):
    """weights: dict with l{1,2}_{wk,wv,wo,w1,w2} and w_out (fp32 numpy)."""
    bf = ml_dtypes.bfloat16
    NJ = (S // P) // 2
    ND = D // P
    SO = S // 2
    VS = V // n_cores
    emb_f = np.ascontiguousarray(emb, dtype=np.float32)
    pe_f = np.asarray(pe, dtype=np.float32)
    scale = 1.0 / np.sqrt(float(D))
    w_bf = {}
    for l in (1, 2):
        w_bf[f"l{l}_wk"] = (np.asarray(weights[f"l{l}_wk"], np.float32) * scale).astype(bf)
        for nm in ("wv", "wo", "w1", "w2"):
            w_bf[f"l{l}_{nm}"] = np.asarray(weights[f"l{l}_{nm}"], np.float32).astype(bf)
    w_out_bf = np.asarray(weights["w_out"], np.float32).astype(bf)

    tokens = np.asarray(tokens)
    in_maps = []
    # scoresT head-block masks: [t_row, q_col] within the diagonal 128-tile;
    # allowed iff q >= t  ->  NEG on the strict lower triangle
    tri = np.tril(np.full((P, P), NEG, np.float32), k=-1)
    for c in range(n_cores):
        b, t = c // 2, c % 2
        own_rows = np.concatenate(
            [np.arange((2 * j + t) * P, (2 * j + t + 1) * P) for j in range(NJ)]
        )
        h0 = (emb_f[tokens[b, own_rows]] + pe_f[own_rows]).astype(np.float32)
        h0T = np.ascontiguousarray(h0.T).reshape(ND, P, SO).astype(bf)
        # head-block mask per stored rank r: global q-tile 2m+t vs key tile
        # 2m+r: t==r -> diagonal triangle; t<r -> fully masked; t>r -> allowed
        maskT = np.zeros((2, P, P), np.float32)
        for r in range(2):
            if t == r:
                maskT[r] = tri
            elif t < r:
                maskT[r] = NEG
        in_map = {
            "h0T": h0T,
            "maskT": maskT.astype(bf),
            "w_out": np.ascontiguousarray(w_out_bf[:, c * VS : (c + 1) * VS]),
        }
        in_map.update(w_bf)
        in_maps.append(in_map)
    return in_maps


_NC_CACHE = {}


def _get_nc(key=(2048, 1024, 4096, 32000, 8)):
    if key not in _NC_CACHE:
        _NC_CACHE[key] = build_nc(*key)
    return _NC_CACHE[key]


def kernel(tokens, emb, pe, l1_wk, l1_wv, l1_wo, l1_w1, l1_w2,
           l2_wk, l2_wv, l2_wo, l2_w1, l2_w2, w_out):
    S = int(np.asarray(tokens).shape[1])
    D = int(np.asarray(emb).shape[1])
    H = int(np.asarray(l1_w1).shape[1])
    V = int(np.asarray(emb).shape[0])
    n_cores = 8
    nc = _get_nc((S, D, H, V, n_cores))
    weights = dict(
        l1_wk=l1_wk, l1_wv=l1_wv, l1_wo=l1_wo, l1_w1=l1_w1, l1_w2=l1_w2,
        l2_wk=l2_wk, l2_wv=l2_wv, l2_wo=l2_wo, l2_w1=l2_w1, l2_w2=l2_w2,
        w_out=w_out,
    )
    in_maps = make_in_maps(tokens, emb, pe, weights, S, D, H, V, n_cores)
    try:
        res = run_bass_kernel_spmd(nc, in_maps, core_ids=list(range(n_cores)))
    except Exception:
        # a previous crashed run can leave the device wedged; one retry
        # (fresh NRT session) clears it
        import os
        os.environ.setdefault("NEURON_RT_RESET_CORES", "1")
        res = run_bass_kernel_spmd(nc, in_maps, core_ids=list(range(n_cores)))
    VS = V // n_cores
    out = np.zeros((np.asarray(tokens).shape[0], V), np.float32)
    for c in range(n_cores):
        out[:, c * VS : (c + 1) * VS] = res.results[c]["logits"]
    return out
